# revision 37
# baseline (speedup 1.0000x reference)
# Causal self-attention (B=4, T=2048, C=1024, H=16, D=64) on 8 TRN2 NeuronCores.
#
# Sharding: core c = (batch b = c//2, head-half g = c%2) -> 8 heads of one batch.
# Each core computes the qkv projection for its head group, causal attention,
# and a rank-512 partial of the output projection. Host sums the two partials
# per batch and adds the constant vector W_proj @ b_v + b_proj (the k-bias is
# dropped: softmax is invariant to it; the v-bias commutes through the convex
# combination).
#
# On-core scheme (all matmuls float32r, scores transposed):
#   qT/kT tiles [128, T] hold two heads (partitions 0-63 / 64-127); S^T tiles
#   [s=128, t=512] come from K=64 matmul pairs packed on the PE array via
#   row groups (base_partition 0/64). exp() needs no max-subtraction
#   (|S| <~ 2 after the 0.125 prescale on Wq). Row sums come free from an
#   appended ones-column on V (M=65 AV matmuls); normalization = DVE
#   reciprocal + gpsimd partition_broadcast + DVE mult.
#   QKV for block n is emitted fused with attention for t-block j=n so the
#   PE-heavy projection work overlaps the ACT-heavy exp work.
#   The causal mask is a single [128,128] triangle (identical for every
#   diagonal s-tile) added only over the 128-wide diagonal band.
import numpy as np

B, T, C, H, D = 4, 2048, 1024, 16, 64
NEG = -30000.0

_NC = None


def _build(reps=1):
    import concourse.bacc as bacc
    import concourse.tile as tile
    from concourse import mybir

    F32R = mybir.dt.float32r
    F32 = mybir.dt.float32
    AF = mybir.ActivationFunctionType
    ALU = mybir.AluOpType

    nc = bacc.Bacc("TRN2", target_bir_lowering=False, debug=False, num_devices=8)
    xT = nc.dram_tensor("xT", [C, T], F32R, kind="ExternalInput")
    wqT = nc.dram_tensor("wqT", [C, 512], F32R, kind="ExternalInput")
    wkT = nc.dram_tensor("wkT", [C, 512], F32R, kind="ExternalInput")
    wvT = nc.dram_tensor("wvT", [C, 512], F32R, kind="ExternalInput")
    wpT = nc.dram_tensor("wpT", [512, C], F32R, kind="ExternalInput")
    bq2 = nc.dram_tensor("bq2", [4, 128], F32, kind="ExternalInput")
    mask = nc.dram_tensor("mask", [128, 128], F32, kind="ExternalInput")
    out = nc.dram_tensor("out", [T, C], F32, kind="ExternalOutput")

    NJ = T // 512  # t blocks
    NS = T // 128  # s tiles

    with tile.TileContext(nc) as tc:
        with (
            tc.tile_pool(name="const", bufs=1) as const,
            tc.tile_pool(name="xq_p", bufs=1) as xq_p,
            tc.tile_pool(name="qt_p", bufs=8) as qt_p,
            tc.tile_pool(name="pt_p", bufs=5) as pt_p,
            tc.tile_pool(name="ot_p", bufs=8) as ot_p,
            tc.tile_pool(name="sm_p", bufs=2) as sm_p,
            tc.tile_pool(name="ob_p", bufs=4) as ob_p,
            tc.tile_pool(name="ps_a", bufs=3, space="PSUM") as ps_a,
            tc.tile_pool(name="ps_st", bufs=3, space="PSUM") as ps_st,
            tc.tile_pool(name="ps_o", bufs=1, space="PSUM") as ps_o,
        ):
            # resident weights / constants
            wq_sb = const.tile([128, 8, 512], F32R)
            wk_sb = const.tile([128, 8, 512], F32R)
            wv_sb = const.tile([128, 8, 512], F32R)
            wp_sb = const.tile([128, 4, C], F32R)
            mk_sb = const.tile([128, 128], F32)
            bq_sb = const.tile([128, 4], F32)
            ones_sb = const.tile([128, 8], F32)
            nc.vector.memset(ones_sb[:], 1.0)

            xTr = xT.rearrange("(kt p) t -> p kt t", p=128)

            def load_x(n):
                xq = []
                for half in range(2):
                    xh = xq_p.tile(
                        [128, 4, 512], F32R,
                        name=f"xq_{n}_{half}", tag=f"xq{half}",
                    )
                    nc.sync.dma_start(
                        xh[:],
                        xTr[:, 4 * half : 4 * half + 4, 512 * n : 512 * (n + 1)],
                    )
                    xq.append(xh)
                return xq

            # DMA priority order: the q-projection of block 0 needs wq + x
            # block 0; k/v weights follow, split so the first halves land
            # just ahead of their first use.
            nc.sync.dma_start(wq_sb[:], wqT.rearrange("(kt p) m -> p kt m", p=128))
            xq_next = load_x(0)
            wkr = wkT.rearrange("(kt p) m -> p kt m", p=128)
            nc.sync.dma_start(wk_sb[:, 0:4, :], wkr[:, 0:4, :])
            nc.sync.dma_start(wk_sb[:, 4:8, :], wkr[:, 4:8, :])
            nc.sync.dma_start(bq_sb[:], bq2.rearrange("m p -> p m"))
            wvr = wvT.rearrange("(kt p) m -> p kt m", p=128)
            nc.sync.dma_start(wv_sb[:, 0:4, :], wvr[:, 0:4, :])
            nc.sync.dma_start(wv_sb[:, 4:8, :], wvr[:, 4:8, :])
            nc.sync.dma_start(mk_sb[:], mask[:, :])
            nc.sync.dma_start(wp_sb[:], wpT.rearrange("(pr p) co -> p pr co", p=128))

            kt = [
                const.tile([128, T], F32R, name=f"kt{i}", tag=f"kt{i}")
                for i in range(4)
            ]
            vt = [
                const.tile([128, 8, 65], F32R, name=f"vt{i}", tag=f"vt{i}")
                for i in range(NS)
            ]

            # warm-up matmuls on resident data: keep the PE array ramped while
            # the first weight/activation DMAs stream in (output never read)
            warm = ps_st.tile([128, 512], F32, tag="st", name="warm_ps")
            for _ in range(20):
                nc.tensor.matmul(
                    warm[0:8, :], ones_sb[:].bitcast(F32R), kt[0][:, 0:512],
                    start=True, stop=True,
                )

            def emit_proj(j, ot2):
                for tt in range(4):
                    for half in range(2):
                        pp = ps_a.tile([128, 512], F32, tag="ps_a", name=f"pp_{j}_{tt}_{half}")
                        for pair in range(4):
                            nc.tensor.matmul(
                                pp[:],
                                ot2[pair][:, 128 * tt : 128 * (tt + 1)],
                                wp_sb[:, pair, 512 * half : 512 * (half + 1)],
                                start=(pair == 0),
                                stop=(pair == 3),
                            )
                        ob = ob_p.tile([128, 512], F32, tag="ob", name=f"ob_{j}_{tt}_{half}")
                        nc.vector.tensor_copy(ob[:], pp[:])
                        nc.sync.dma_start(
                            out[
                                512 * j + 128 * tt : 512 * j + 128 * (tt + 1),
                                512 * half : 512 * (half + 1),
                            ],
                            ob[:],
                        )

            def emit_qk_group(rep, n, xq, qt, mt):
                if True:
                    psq = ps_a.tile([128, 512], F32, tag="ps_a", name=f"psq_{rep}_{n}_{mt}")
                    for k in range(8):
                        nc.tensor.matmul(
                            psq[:],
                            wq_sb[:, k, 128 * mt : 128 * (mt + 1)],
                            xq[k // 4][:, k % 4, :],
                            start=(k == 0),
                            stop=(k == 7),
                        )
                    q_tile = qt_p.tile(
                        [128, 512], F32R, name=f"qt_{rep}_{mt}_{n}", tag="qt"
                    )
                    qt[(mt, n)] = q_tile
                    nc.vector.tensor_scalar_add(
                        q_tile[:], psq[:], bq_sb[:, mt : mt + 1]
                    )
                    psk = ps_a.tile([128, 512], F32, tag="ps_a", name=f"psk_{rep}_{n}_{mt}")
                    for k in range(8):
                        nc.tensor.matmul(
                            psk[:],
                            wk_sb[:, k, 128 * mt : 128 * (mt + 1)],
                            xq[k // 4][:, k % 4, :],
                            start=(k == 0),
                            stop=(k == 7),
                        )
                    nc.vector.tensor_copy(
                        kt[mt][:, 512 * n : 512 * (n + 1)], psk[:]
                    )
            def emit_v(rep, n, xq):
                for tt in range(4):
                    psv = ps_a.tile([128, 512], F32, tag="ps_a", name=f"psv_{rep}_{n}_{tt}")
                    for k in range(8):
                        nc.tensor.matmul(
                            psv[:],
                            xq[k // 4][:, k % 4, 128 * tt : 128 * (tt + 1)],
                            wv_sb[:, k, :],
                            start=(k == 0),
                            stop=(k == 7),
                        )
                    si = 4 * n + tt
                    nc.vector.tensor_copy(
                        vt[si][:, :, 0:64],
                        psv.rearrange("p (h d) -> p h d", d=64),
                    )
                    nc.vector.tensor_copy(vt[si][:, :, 64], ones_sb[:])

            def att_segment(rep, j, pair, qt, ot2, fast_tail):
                oaug = [
                    ps_o.tile(
                        [65, 512], F32,
                        name=f"oaug_{rep}_{j}_{pair}_{h}", tag=f"ps_o{h}",
                    )
                    for h in range(2)
                ]
                ns_live = 4 * (j + 1)
                for si in range(ns_live):
                    r = si - 4 * j
                    off = 128 * r if r > 0 else 0
                    sts = []
                    for h in range(2):
                        st = ps_st.tile([128, 512], F32, tag="st",
                                        name=f"st_{rep}_{j}_{pair}_{si}_{h}")
                        nc.tensor.matmul(
                            st[:, off:],
                            kt[pair][
                                64 * h : 64 * h + 64,
                                128 * si : 128 * si + 128,
                            ],
                            qt[(pair, j)][64 * h : 64 * h + 64, off:],
                            start=True,
                            stop=True,
                        )
                        sts.append(st)
                    for h in range(2):
                        if r >= 0:
                            nc.vector.tensor_tensor(
                                sts[h][:, off : off + 128],
                                sts[h][:, off : off + 128],
                                mk_sb[:],
                                ALU.add,
                            )
                        pt = pt_p.tile([128, 512], F32R, tag="pt",
                                       name=f"pt_{rep}_{j}_{pair}_{si}_{h}")
                        nc.scalar.activation(pt[:, off:], sts[h][:, off:], AF.Exp)
                        nc.tensor.matmul(
                            oaug[h][:, off:],
                            vt[si][:, 2 * pair + h, :],
                            pt[:, off:],
                            start=(si == 0),
                            stop=(si == ns_live - 1),
                        )
                o_tile = ot_p.tile(
                    [128, 512], F32R, name=f"ot_{rep}_{pair}_{j}", tag="ot"
                )
                ot2.append(o_tile)
                for h in range(2):
                    bc = sm_p.tile([64, 512], F32, tag="bc", bufs=2,
                                   name=f"bc_{rep}_{j}_{pair}_{h}")
                    if fast_tail:
                        # nothing reuses this PSUM bank afterwards: skip the
                        # decoupling copy, shortest chain into the final proj
                        src = oaug[h]
                    else:
                        # single copy frees the PSUM bank; normalize runs on
                        # the SBUF copy off the PE critical path
                        oa = sm_p.tile([65, 512], F32, tag="oa",
                                       name=f"oa_{rep}_{j}_{pair}_{h}")
                        nc.vector.tensor_copy(oa[:], oaug[h][:])
                        src = oa
                    rec = sm_p.tile([1, 512], F32, tag="rec", bufs=2,
                                    name=f"rec_{rep}_{j}_{pair}_{h}")
                    nc.vector.reciprocal(rec[:], src[64:65, :])
                    nc.gpsimd.partition_broadcast(bc[:], rec[:])
                    nc.vector.tensor_tensor(
                        o_tile[64 * h : 64 * h + 64, :],
                        src[0:64, :],
                        bc[:],
                        ALU.mult,
                    )

            # Schedule: att(j) is emitted one iteration late so its ACT-heavy
            # exp work co-schedules with the next block's PE-dense QKV; the
            # final two attentions are pair-interleaved so the PE always has a
            # second independent stream while exp limits the other.
            for rep in range(reps):
                qt = {}
                ots = {j: [] for j in range(NJ)}
                for n in range(NJ):
                    xq = xq_next
                    # weave attention pair-segments between the QKV groups
                    # they depend on so exp work reaches ACT early; the
                    # previous block's projection trails as PE stall filler
                    emit_qk_group(rep, n, xq, qt, 0)
                    emit_qk_group(rep, n, xq, qt, 1)
                    emit_v(rep, n, xq)
                    if n + 1 < NJ:
                        xq_next = load_x(n + 1)
                    elif rep + 1 < reps:
                        xq_next = load_x(0)
                    last = rep + 1 == reps and n == NJ - 1
                    att_segment(rep, n, 0, qt, ots[n], False)
                    emit_qk_group(rep, n, xq, qt, 2)
                    att_segment(rep, n, 1, qt, ots[n], False)
                    emit_qk_group(rep, n, xq, qt, 3)
                    att_segment(rep, n, 2, qt, ots[n], False)
                    att_segment(rep, n, 3, qt, ots[n], last)
                    if n >= 1:
                        emit_proj(n - 1, ots[n - 1])
                emit_proj(NJ - 1, ots[NJ - 1])
    nc.compile()
    return nc


def _get_nc():
    global _NC
    if _NC is None:
        _NC = _build()
    return _NC


def _host_mask():
    i = np.arange(128)[:, None]
    j = np.arange(128)[None, :]
    return np.where(j >= i, 0.0, NEG).astype(np.float32)


def _in_maps(x, W_attn, b_attn, W_proj):
    Wq, Wk, Wv = W_attn[0:C], W_attn[C : 2 * C], W_attn[2 * C : 3 * C]
    mask = _host_mask()
    g_in = []
    for g in range(2):
        sl = slice(512 * g, 512 * (g + 1))
        g_in.append(
            dict(
                wqT=np.ascontiguousarray(Wq[sl].T) * 0.125,
                wkT=np.ascontiguousarray(Wk[sl].T),
                wvT=np.ascontiguousarray(Wv[sl].T),
                wpT=np.ascontiguousarray(W_proj[:, sl].T),
                bq2=(b_attn[sl] * 0.125).reshape(4, 128).astype(np.float32),
                mask=mask,
            )
        )
    xTs = [np.ascontiguousarray(x[b].T) for b in range(B)]
    return [dict(xT=xTs[c // 2], **g_in[c % 2]) for c in range(8)]


def kernel(x, W_attn, b_attn, W_proj, b_proj):
    from concourse.bass_utils import run_bass_kernel_spmd

    x = np.asarray(x, dtype=np.float32)
    W_attn = np.asarray(W_attn, dtype=np.float32)
    b_attn = np.asarray(b_attn, dtype=np.float32)
    W_proj = np.asarray(W_proj, dtype=np.float32)
    b_proj = np.asarray(b_proj, dtype=np.float32)

    nc = _get_nc()
    in_maps = _in_maps(x, W_attn, b_attn, W_proj)
    res = run_bass_kernel_spmd(nc, in_maps, core_ids=list(range(8)))

    cvec = (W_proj @ b_attn[2 * C : 3 * C] + b_proj).astype(np.float32)
    y = np.empty((B, T, C), np.float32)
    for b in range(B):
        y[b] = res.results[2 * b]["out"] + res.results[2 * b + 1]["out"] + cvec
    return y
